# revision 1
# baseline (speedup 1.0000x reference)
"""MoE (B=4,T=4096,C=1024,E=8,top2) Trainium2 kernel — 8-core data-parallel.

Strategy: shard tokens across the 8 cores (2048 tokens/core); every core
computes the gate + all 8 experts densely on its tokens in fp32r matmuls,
with the top-2 softmax combine applied as per-token scalars (token-major
layout all the way; zero transposes).

Self-contained: only numpy + concourse imports, shapes hardcoded.
"""
from contextlib import ExitStack

import numpy as np

import concourse.bass as bass
import concourse.tile as tile
from concourse import bacc, mybir

P = 128
B, T, C, E = 4, 4096, 1024, 8
H = 4 * C
NCORES = 8
NTOK = (B * T) // NCORES   # 2048 tokens per core
NT = 1024                  # token block
HB = 512                   # hidden block
F32 = mybir.dt.float32
F32R = mybir.dt.float32r
AF = mybir.ActivationFunctionType
OP = mybir.AluOpType

CC = C // P        # 8 C chunks
NB = NTOK // NT    # token blocks per core
NTC = NT // P      # 8 token chunks per block
NHB = H // HB      # 8 hidden blocks
HC = HB // P       # 4 h chunks per block
CN = C // 512      # GEMM2 n-tiles
TN = NT // 512     # GEMM1 n-tiles


def _r(ap):
    return ap.bitcast(F32R)


def build_nc(rep: int = 1):
    nc = bacc.Bacc(target_bir_lowering=False)
    xT = nc.dram_tensor("xT", [C, NTOK], F32, kind="ExternalInput")
    gw = nc.dram_tensor("gw", [C, E], F32, kind="ExternalInput")
    gb = nc.dram_tensor("gb", [1, E], F32, kind="ExternalInput")
    w1 = nc.dram_tensor("w1", [E, C, H], F32, kind="ExternalInput")
    b1 = nc.dram_tensor("b1", [E, H], F32, kind="ExternalInput")
    w2 = nc.dram_tensor("w2", [E, H, C], F32, kind="ExternalInput")
    b2 = nc.dram_tensor("b2", [E, C], F32, kind="ExternalInput")
    out = nc.dram_tensor("out", [NTOK, C], F32, kind="ExternalOutput")

    with tile.TileContext(nc) as tc, ExitStack() as ctx:
        sing = ctx.enter_context(tc.tile_pool(name="sing", bufs=1))
        xt_pool = ctx.enter_context(tc.tile_pool(name="xt", bufs=1))
        w1_pool = ctx.enter_context(tc.tile_pool(name="w1p", bufs=2))
        w2_pool = ctx.enter_context(tc.tile_pool(name="w2p", bufs=2))
        ht_pool = ctx.enter_context(tc.tile_pool(name="ht", bufs=2))
        acc_pool = ctx.enter_context(tc.tile_pool(name="accp", bufs=1))
        small = ctx.enter_context(tc.tile_pool(name="small", bufs=2))
        comb_pool = ctx.enter_context(tc.tile_pool(name="comb", bufs=2))
        ps_g = ctx.enter_context(tc.tile_pool(name="psg", bufs=1, space="PSUM"))
        ps1 = ctx.enter_context(tc.tile_pool(name="ps1", bufs=3, space="PSUM"))
        ps2 = ctx.enter_context(tc.tile_pool(name="ps2", bufs=4, space="PSUM"))

        # one-time loads
        gw_sb = sing.tile([P, CC, E], F32R)
        nc.sync.dma_start(
            out=gw_sb, in_=_r(gw[:].rearrange("(cc p) e -> p cc e", p=P)))
        gb_sb = sing.tile([P, E], F32)
        nc.sync.dma_start(out=gb_sb, in_=gb[:].to_broadcast([P, E]))
        b1_sb = sing.tile([P, E, H // P], F32)
        for e in range(E):
            nc.sync.dma_start(
                out=b1_sb[:, e, :], in_=b1[e, :].rearrange("(j p) -> p j", p=P))
        b2_sb = sing.tile([1, E * C], F32R)
        nc.sync.dma_start(
            out=b2_sb, in_=_r(b2[:].rearrange("e c -> (e c)")[None, :]))
        ones = sing.tile([1, P], F32)
        nc.vector.memset(ones[:], 1.0)

        for _ in range(rep):
            for tb in range(NB):
                xt = xt_pool.tile([P, CC, NT], F32R)
                nc.sync.dma_start(
                    out=xt,
                    in_=_r(xT[:, tb * NT:(tb + 1) * NT]
                           .rearrange("(cc p) t -> p cc t", p=P)))

                # ---- gate: logits -> top2 -> softmax -> dense comb [tok, E]
                comb = comb_pool.tile([P, NTC, E], F32)
                for tcb in range(NTC):
                    psg = ps_g.tile([P, E], F32)
                    for cc in range(CC):
                        nc.tensor.matmul(
                            out=psg[:],
                            lhsT=xt[:, cc, tcb * P:(tcb + 1) * P],
                            rhs=gw_sb[:, cc, :],
                            start=(cc == 0), stop=(cc == CC - 1))
                    logits = small.tile([P, E], F32, tag="logits")
                    nc.vector.tensor_add(out=logits[:], in0=psg[:], in1=gb_sb[:])
                    top8 = small.tile([P, E], F32, tag="top8")
                    nc.vector.max(out=top8[:], in_=logits[:])
                    d0 = small.tile([P, 2], F32, tag="d0")
                    nc.vector.tensor_sub(
                        out=d0[:, 0:1], in0=top8[:, 1:2], in1=top8[:, 0:1])
                    nc.vector.tensor_sub(
                        out=d0[:, 1:2], in0=top8[:, 0:1], in1=top8[:, 1:2])
                    wab = small.tile([P, 2], F32, tag="wab")
                    # wab[:,0] = sigmoid(v1-v0) = weight of 2nd; wab[:,1] = 1st
                    nc.scalar.activation(out=wab[:], in_=d0[:], func=AF.Sigmoid)
                    m1 = small.tile([P, E], F32, tag="m1")
                    nc.vector.tensor_scalar(
                        out=m1[:], in0=logits[:],
                        scalar1=top8[:, 0:1], scalar2=wab[:, 1:2],
                        op0=OP.is_equal, op1=OP.mult)
                    m2 = small.tile([P, E], F32, tag="m2")
                    nc.vector.tensor_scalar(
                        out=m2[:], in0=logits[:],
                        scalar1=top8[:, 1:2], scalar2=wab[:, 0:1],
                        op0=OP.is_equal, op1=OP.mult)
                    nc.vector.tensor_add(
                        out=comb[:, tcb, :], in0=m1[:], in1=m2[:])

                # ---- experts
                acc = acc_pool.tile([P, NTC, C], F32)
                nc.vector.memset(acc[:], 0.0)
                for e in range(E):
                    for hb in range(NHB):
                        w1_t = w1_pool.tile([P, CC, HB], F32R)
                        nc.sync.dma_start(
                            out=w1_t,
                            in_=_r(w1[e, :, hb * HB:(hb + 1) * HB]
                                   .rearrange("(cc p) h -> p cc h", p=P)))
                        w2_t = w2_pool.tile([P, HC, C], F32R)
                        nc.sync.dma_start(
                            out=w2_t,
                            in_=_r(w2[e, hb * HB:(hb + 1) * HB, :]
                                   .rearrange("(hc p) c -> p hc c", p=P)))
                        ht = ht_pool.tile([P, HC, NT], F32R)
                        # GEMM1: hT[hb] = relu(w1[:,hb]^T @ xT + b1)
                        for hc in range(HC):
                            for tn in range(TN):
                                ps = ps1.tile([P, 512], F32)
                                for cc in range(CC):
                                    nc.tensor.matmul(
                                        out=ps[:],
                                        lhsT=w1_t[:, cc, hc * P:(hc + 1) * P],
                                        rhs=xt[:, cc, tn * 512:(tn + 1) * 512],
                                        start=(cc == 0), stop=(cc == CC - 1))
                                nc.scalar.activation(
                                    out=ht[:, hc, tn * 512:(tn + 1) * 512],
                                    in_=ps[:], func=AF.Relu,
                                    bias=b1_sb[:, e, hb * HC + hc:hb * HC + hc + 1],
                                    scale=1.0)
                        # GEMM2 partial: y[tok, C] += hT[hb]^T @ w2[hb]
                        for tcb in range(NTC):
                            for cn in range(CN):
                                ps = ps2.tile([P, 512], F32)
                                for hc in range(HC):
                                    nc.tensor.matmul(
                                        out=ps[:],
                                        lhsT=ht[:, hc, tcb * P:(tcb + 1) * P],
                                        rhs=w2_t[:, hc, cn * 512:(cn + 1) * 512],
                                        start=(hc == 0),
                                        stop=(hc == HC - 1 and hb != 0))
                                if hb == 0:
                                    # fold b2 in as a rank-1 (K=1) matmul
                                    nc.tensor.matmul(
                                        out=ps[:],
                                        lhsT=_r(ones[:]),
                                        rhs=b2_sb[0:1,
                                                  e * C + cn * 512:
                                                  e * C + cn * 512 + 512],
                                        start=False, stop=True)
                                sl = acc[:, tcb, cn * 512:(cn + 1) * 512]
                                nc.vector.scalar_tensor_tensor(
                                    out=sl, in0=ps[:],
                                    scalar=comb[:, tcb, e:e + 1], in1=sl,
                                    op0=OP.mult, op1=OP.add)
                nc.sync.dma_start(
                    out=out[tb * NT:(tb + 1) * NT, :]
                        .rearrange("(tcb p) c -> p tcb c", p=P),
                    in_=acc[:])
    nc.finalize()
    return nc


def make_in_maps(inputs):
    x = np.ascontiguousarray(np.asarray(inputs["x"], dtype=np.float32))
    xf = x.reshape(B * T, C)
    gw = np.ascontiguousarray(np.asarray(inputs["gate_w"], np.float32))
    gb = np.ascontiguousarray(
        np.asarray(inputs["gate_b"], np.float32).reshape(1, E))
    w1 = np.ascontiguousarray(np.asarray(inputs["w1"], np.float32))
    b1 = np.ascontiguousarray(np.asarray(inputs["b1"], np.float32))
    w2 = np.ascontiguousarray(np.asarray(inputs["w2"], np.float32))
    b2 = np.ascontiguousarray(np.asarray(inputs["b2"], np.float32))
    in_maps = []
    for c in range(NCORES):
        shard = xf[c * NTOK:(c + 1) * NTOK]            # [NTOK, C]
        xTs = np.ascontiguousarray(shard.T)            # [C, NTOK]
        in_maps.append({
            "xT": xTs, "gw": gw, "gb": gb,
            "w1": w1, "b1": b1, "w2": w2, "b2": b2,
        })
    return in_maps


def assemble(results):
    outs = [results[c]["out"] for c in range(NCORES)]
    return np.concatenate(outs, axis=0).reshape(B, T, C)


def kernel(**inputs) -> np.ndarray:
    from concourse import bass2jax
    nc = build_nc()
    in_maps = make_in_maps(inputs)
    results = bass2jax.run_bass_via_pjrt(nc, in_maps, n_cores=NCORES)
    return assemble(results)
